# revision 15
# baseline (speedup 1.0000x reference)
"""BitLinear inference kernel for Trainium2, SPMD over 8 NeuronCores.

Reference computation (x [B, T, D] f32, kernel [D, F] f32):
  x_norm  = x * rsqrt(mean(x^2, -1) + 1e-5)
  x_scale = 127 / clip(max|x_norm|, 1e-5)          (per row)
  x_quant = round(x_norm * x_scale).clip(-128,127) / x_scale
  w_scale = mean|kernel|.clip(1e-5)
  w_quant = sign(kernel - mean(kernel)) * w_scale
  out     = (x_quant @ w_quant) / w_scale / x_scale

Algebra: w_scale cancels exactly, and with
  q    = round(x * 127/max|x|)   (row scale cancels inside the quant)
  s    = sign(kernel - mean(kernel))  (+-1)
  out  = (q @ s) * (max|x|^2 / (mean(x^2)+1e-5)) / 127^2   per row.
q and s are exactly representable in bf16 and the PE accumulates in
fp32, so the matmul is exact integer arithmetic.

Sharding: 2D grid, 4 data-groups x 2 feature-groups.  Core (dg, fg)
computes batches [2*dg, 2*dg+2) for output features [2048*fg, 2048*(fg+1)).
Its sign slice s[:, fslice] is [4096, 2048] bf16 = 128 KiB/partition,
fully resident in SBUF: produced once by the Activation engine straight
from streamed w columns (no DRAM round-trip, no re-reads).  The global
w mean comes from per-core disjoint 512-row partial column sums +
a tiny AllReduce, so sign production starts ~90us in.  After that the
kernel is a single sweep over 64 token blocks, PE-bound end to end.
"""

import re
from contextlib import ExitStack

import numpy as np

import concourse.bass as bass
import concourse.mybir as mybir
import concourse.tile as tile
from concourse.tile import ScopedClock, VectorClock


# ---------------------------------------------------------------------------
# The walrus build in this container only accepts a single sync-wait per
# Drain instruction; TileContext's tail drain carries one wait per live
# semaphore.  Split it into one drain per semaphore.
# ---------------------------------------------------------------------------
def _drain_and_barrier_split(self, tick_clock, wait_clock):
    m = re.search(r"VectorClock\(\[([^\]]*)\]\)", repr(tick_clock.global_clock))
    vals = [int(v) for v in m.group(1).split(",")]
    procs = [(i, v) for i, v in enumerate(vals) if v > 0]
    for i, v in procs or [(0, 0)]:
        sub = VectorClock()
        sub.require_at_least(i, v)
        drain_inst = self.nc.sync.drain()
        wait_clock.add_sem_waits(drain_inst.ins, ScopedClock({None: sub}))

    self.nc.all_engine_barrier()
    assert self.sems is not None
    popped = self.nc._tile_sem_poison_stack.pop()
    assert popped is self._sem_poison
    self.nc.clear_and_free_semaphores(list(self.sems.allocated().values()))
    self.nc.all_engine_barrier()


def install_drain_patch():
    tile.TileContext._drain_and_barrier = _drain_and_barrier_split


def split_multi_waits(nc: bass.Bass, max_waits: int = 1):
    """The walrus in this container accepts at most one sync-wait per
    instruction.  Hoist extra waits onto NoOps injected just before the
    instruction on the same engine (engines execute their stream in order,
    so waiting on A then B sequentially == waiting on both)."""
    n_split = 0
    for fn in nc.m.functions:
        for bb in fn.blocks:
            insts = bb.instructions
            if not any(
                ins.sync_info is not None and len(ins.sync_info.on_wait or []) > max_waits
                for ins in insts
            ):
                continue
            out = []
            for ins in insts:
                si = ins.sync_info
                if si is not None and len(si.on_wait or []) > max_waits:
                    waits = list(si.on_wait)
                    for j, w in enumerate(waits[:-max_waits]):
                        nop = mybir.InstNoOp(name=f"{ins.name}-wsplit{j}", ins=[], outs=[])
                        nop.engine = ins.engine
                        nop.sync_info = mybir.SyncInfo(on_wait=[w], on_update=[])
                        nc.register_instruction(nop, overwrite=True)
                        out.append(nop)
                    ins.sync_info = mybir.SyncInfo(
                        on_wait=waits[-max_waits:], on_update=list(si.on_update or [])
                    )
                    n_split += 1
                out.append(ins)
            bb.instructions = out
    return n_split


MAGIC = float(1.5 * 2.0**23)  # keeps v+MAGIC in [2^23, 2^24) for |v| <= 2^22 -> RNE to integer
F32 = mybir.dt.float32
BF16 = mybir.dt.bfloat16
P = 128


def build_bitlinear(nc: bass.Bass, T=8192, D=4096, FL=2048, FC=512, WR=512, world=8):
    """Per-core program: x [T, D] f32, w [D, FL] f32 (this core's feature
    column slice), wslice [WR, D] f32 (this core's disjoint row slice of the
    full kernel, for the global-mean partial sum) -> out [T, FL] f32.

    The full [D, FL] sign slice stays resident in SBUF (bf16, 128
    KiB/partition); the main loop is a single sweep over T/128 token
    blocks with 4 PSUM banks of 512 features each accumulating over
    D/128 = 32 contraction slices."""
    AF = mybir.ActivationFunctionType
    KB = D // P          # contraction slices
    NTB = T // P         # 128-token blocks
    NFC = FL // FC       # feature chunks (PSUM-bank sized)
    SUB = 512            # bn_stats subgroup width
    NSUB = D // SUB
    WRB = WR // P        # row-blocks in wslice

    x_in = nc.dram_tensor("x", [T, D], F32, kind="ExternalInput")
    w_in = nc.dram_tensor("w", [D, FL], F32, kind="ExternalInput")
    ws_in = nc.dram_tensor("wslice", [WR, D], F32, kind="ExternalInput")
    out = nc.dram_tensor("out", [T, FL], F32, kind="ExternalOutput")

    with tile.TileContext(nc) as tc, ExitStack() as ctx:
        xp = ctx.enter_context(tc.tile_pool(name="xp", bufs=2))
        qbp = ctx.enter_context(tc.tile_pool(name="qbp", bufs=1))
        qtp = ctx.enter_context(tc.tile_pool(name="qtp", bufs=2))
        w2p = ctx.enter_context(tc.tile_pool(name="w2p", bufs=2))
        w3p = ctx.enter_context(tc.tile_pool(name="w3p", bufs=1))
        stg = ctx.enter_context(tc.tile_pool(name="stg", bufs=3))
        st = ctx.enter_context(tc.tile_pool(name="st", bufs=4))
        postp = ctx.enter_context(tc.tile_pool(name="postp", bufs=4))
        singles = ctx.enter_context(tc.tile_pool(name="singles", bufs=1))
        psmm = ctx.enter_context(tc.tile_pool(name="psmm", bufs=7, space="PSUM"))
        psw = ctx.enter_context(tc.tile_pool(name="psw", bufs=1, space="PSUM"))
        dram = ctx.enter_context(tc.tile_pool(name="dram", bufs=1, space="DRAM"))

        # ---- W pass 1: global mean via per-core partial sums + AllReduce ----
        # Contiguous full-width row-block reads (2 MiB DMAs) of this core's
        # disjoint 512-row slice of the full kernel.
        colsum = singles.tile([P, WRB], F32)
        for rb in range(WRB):
            wt = xp.tile([P, D], F32, tag="xt")
            nc.sync.dma_start(out=wt, in_=ws_in[rb * P:(rb + 1) * P, :])
            nc.vector.reduce_sum(
                out=colsum[:, rb : rb + 1], in_=wt, axis=mybir.AxisListType.X
            )
        if world > 1:
            cc_in = dram.tile([P, WRB], F32, name="cc_in")
            cc_out = dram.tile([P, WRB], F32, name="cc_out", addr_space="Shared")
            nc.gpsimd.dma_start(out=cc_in[:, :], in_=colsum)
            nc.gpsimd.collective_compute(
                "AllReduce",
                mybir.AluOpType.add,
                replica_groups=[list(range(world))],
                ins=[cc_in[:, :]],
                outs=[cc_out[:, :]],
            )
            colsum_all = st.tile([P, WRB], F32)
            nc.gpsimd.dma_start(out=colsum_all, in_=cc_out[:, :])
        else:
            colsum_all = colsum
        rowsum = st.tile([P, 1], F32)
        nc.vector.reduce_sum(out=rowsum, in_=colsum_all, axis=mybir.AxisListType.X)
        ones_sq = singles.tile([P, P], F32)
        nc.vector.memset(ones_sq, 1.0)
        magic_bias = singles.tile([P, 1], F32)
        nc.vector.memset(magic_bias, MAGIC)
        # single matmul reduces the partition dim AND broadcasts the total to
        # all 128 partitions: out[m] = sum_k ones[k,m] * rowsum[k]
        ps_bc = psw.tile([P, 1], F32)
        nc.tensor.matmul(ps_bc, lhsT=ones_sq, rhs=rowsum, start=True, stop=True)
        neg_wmean = singles.tile([P, 1], F32)
        nc.scalar.activation(neg_wmean, ps_bc, AF.Copy, bias=0.0, scale=-1.0 / (D * D))

        # ---- sign production: s = sign(w - mean), straight into SBUF ----
        # One persistent bf16 tile per 512-wide feature chunk.  The w column
        # reads (2 KiB contiguous per row) ride the ACT HWDGE ring and have no
        # dependency on the mean, so they prefetch from t=0; the Sign
        # activations fire as soon as the AllReduced mean lands.  Chunk 0
        # completes first so the first token block's matmuls start while
        # chunks 1-3 are still being produced.
        s_sb = [
            singles.tile([P, KB, FC], BF16, tag=f"ssb{fc}", name=f"ssb{fc}")
            for fc in range(NFC)
        ]
        # ACT produces chunks 0..NFC-2 (Sign activation); the otherwise-idle
        # Pool engine produces the last chunk concurrently via
        # b = ((w + -mean) >= 0) then s = 2b - 1.  Tiles span TWO k-slices
        # ([P, 2, FC]): the producer stream is paced by tile-pool WAR
        # semaphores at ~3us per link, so halving the link count halves the
        # production drain time.
        for fc in range(NFC - 1):
            for rb2 in range(KB // 2):
                wt2 = w2p.tile([P, 2, FC], F32, tag="wtile")
                nc.scalar.dma_start(
                    out=wt2,
                    in_=w_in[
                        rb2 * 2 * P:(rb2 + 1) * 2 * P, fc * FC:(fc + 1) * FC
                    ].rearrange("(j p) c -> p j c", p=P),
                )
                nc.scalar.activation(
                    out=s_sb[fc][:, 2 * rb2:2 * rb2 + 2, :], in_=wt2, func=AF.Sign,
                    bias=neg_wmean, scale=1.0,
                )
        for fc in range(NFC - 1, NFC):
            for rb2 in range(KB // 2):
                wt3 = w3p.tile([P, 2, FC], F32, tag="wtile2")
                nc.gpsimd.dma_start(
                    out=wt3,
                    in_=w_in[
                        rb2 * 2 * P:(rb2 + 1) * 2 * P, fc * FC:(fc + 1) * FC
                    ].rearrange("(j p) c -> p j c", p=P),
                )
                nc.gpsimd.tensor_scalar(
                    out=s_sb[fc][:, 2 * rb2:2 * rb2 + 2, :], in0=wt3,
                    scalar1=neg_wmean, scalar2=0.0,
                    op0=mybir.AluOpType.add, op1=mybir.AluOpType.is_ge,
                )
                nc.gpsimd.tensor_scalar(
                    out=s_sb[fc][:, 2 * rb2:2 * rb2 + 2, :],
                    in0=s_sb[fc][:, 2 * rb2:2 * rb2 + 2, :],
                    scalar1=2.0, scalar2=-1.0,
                    op0=mybir.AluOpType.mult, op1=mybir.AluOpType.add,
                )

        # ---- main loop: single sweep over token blocks ----
        # Evacuations are emitted one block LATE (after block tb's pre-chain
        # and before its matmuls): when the DVE/ACT queues reach evac(tb-1),
        # its matmuls have long retired, so the wait never head-of-line
        # blocks the next block's stats/round chain on those engines.
        pending = None  # (psum tiles, post, tb) awaiting evacuation

        def flush_evacs():
            nonlocal pending
            if pending is None:
                return
            pss, ppost, ptb = pending
            for i, (fc, ps) in enumerate(pss):
                so = stg.tile([P, FC], F32, tag="so")
                if i % 2 == 0:
                    nc.vector.tensor_scalar_mul(so, ps, ppost)
                else:
                    nc.scalar.activation(
                        out=so, in_=ps, func=AF.Copy, bias=0.0, scale=ppost
                    )
                nc.sync.dma_start(
                    out=out[ptb * P:(ptb + 1) * P, fc * FC:(fc + 1) * FC], in_=so
                )
            pending = None

        for tb in range(NTB):
            xt = xp.tile([P, D], F32, tag="xt")
            nc.sync.dma_start(out=xt, in_=x_in[tb * P:(tb + 1) * P, :])

            # mean(x^2) via bn_stats (no main output needed)
            stats6 = st.tile([P, NSUB, 6], F32)
            for i in range(NSUB):
                nc.vector.bn_stats(out=stats6[:, i, :], in_=xt[:, i * SUB:(i + 1) * SUB])
            mv = st.tile([P, 2], F32)
            nc.vector.bn_aggr(out=mv, in_=stats6)
            msq = st.tile([P, 1], F32)
            nc.vector.tensor_mul(msq, mv[:, 0:1], mv[:, 0:1])
            v0 = st.tile([P, 1], F32)
            nc.vector.tensor_add(v0, msq, mv[:, 1:2])
            v1 = st.tile([P, 1], F32)
            nc.vector.tensor_scalar_add(v1, v0, 1e-5)
            r2 = st.tile([P, 1], F32)
            nc.vector.reciprocal(r2, v1)

            am = st.tile([P, 1], F32)
            nc.vector.tensor_reduce(
                out=am, in_=xt, axis=mybir.AxisListType.X,
                op=mybir.AluOpType.max, apply_absolute_value=True,
            )
            am2 = st.tile([P, 1], F32)
            nc.vector.tensor_mul(am2, am, am)
            a2 = st.tile([P, 1], F32)
            nc.vector.tensor_mul(a2, am2, r2)
            post = postp.tile([P, 1], F32)
            nc.vector.tensor_scalar(
                out=post, in0=a2, scalar1=1e-10, scalar2=1.0 / (127.0 * 127.0),
                op0=mybir.AluOpType.max, op1=mybir.AluOpType.mult,
            )
            w1 = st.tile([P, 1], F32)
            nc.vector.tensor_scalar(
                out=w1, in0=am, scalar1=1e-30, scalar2=1.0 / 127.0,
                op0=mybir.AluOpType.max, op1=mybir.AluOpType.mult,
            )
            cc = st.tile([P, 1], F32)
            nc.vector.reciprocal(cc, w1)

            # q = round(x * c) via the magic-number trick (RNE)
            nc.scalar.activation(out=xt, in_=xt, func=AF.Identity, bias=magic_bias, scale=cc)
            qb = qbp.tile([P, D], BF16)
            nc.vector.tensor_scalar_add(qb, xt, -MAGIC)

            # one batched xbar transpose per block: qT[p,k,t] = qb[t, k*P+p].
            # Stays on the ACT ring: moving it to the sync ring puts its
            # wait-on-sub entry ahead of the next x load (in-order ring) and
            # inflates the steady-state cadence.
            qT = qtp.tile([P, KB, P], BF16, tag="qT")
            nc.scalar.dma_start_transpose(out=qT, in_=qb[:, :])

            # previous block's evacuations go in front of this block's
            # matmuls; their waits are already satisfied
            flush_evacs()

            # chunk order matches production readiness: 0 (ACT first), 3
            # (Pool, concurrent), then 1, 2 (ACT) — so early blocks always
            # have a ready chunk to chew on while production drains
            pss = []
            for fc in (0, NFC - 1, 1, 2) if NFC == 4 else range(NFC):
                ps = psmm.tile([P, FC], F32, tag="ps", name="ps")
                for k in range(KB):
                    nc.tensor.matmul(
                        ps,
                        lhsT=qT[:, k, :],
                        rhs=s_sb[fc][:, k, :],
                        start=(k == 0),
                        stop=(k == KB - 1),
                    )
                pss.append((fc, ps))
            pending = (pss, post, tb)
        flush_evacs()
    return nc


_N_CORES = 8
_BATCH = 8
_SEQ = 4096
_D = 4096
_F = 4096
_DG = 4            # data groups (batch pairs)
_FG = 2            # feature groups
_BPC = _BATCH // _DG      # batches per core
_T = _BPC * _SEQ          # tokens per core
_FL = _F // _FG           # features per core
_WR = _D // _N_CORES      # wslice rows per core


def _ensure_axon_hooks_module():
    """bass_utils imports antenv.axon_hooks when BASS_TRACE is set; the
    module is absent in this image.  Provide a stub so tracing degrades
    gracefully instead of crashing (a real hook may already be installed)."""
    import sys
    import types

    try:
        import antenv.axon_hooks  # noqa: F401
    except ImportError:
        mod = types.ModuleType("antenv.axon_hooks")
        mod._hook = None
        mod.set_axon_ntff_profile_hook = lambda h: setattr(mod, "_hook", h)
        mod.get_axon_ntff_profile_hook = lambda: mod._hook
        sys.modules["antenv.axon_hooks"] = mod


def kernel(x: np.ndarray, kernel: np.ndarray) -> np.ndarray:
    from concourse.bass_utils import run_bass_kernel_spmd

    _ensure_axon_hooks_module()
    install_drain_patch()
    nc = bass.Bass()
    build_bitlinear(nc, T=_T, D=_D, FL=_FL, FC=512, WR=_WR, world=_N_CORES)
    split_multi_waits(nc)

    x = np.ascontiguousarray(np.asarray(x, dtype=np.float32))
    w = np.ascontiguousarray(np.asarray(kernel, dtype=np.float32))
    assert x.shape == (_BATCH, _SEQ, _D) and w.shape == (_D, _F)

    in_maps = []
    for core in range(_N_CORES):
        dg, fg = divmod(core, _FG)
        in_maps.append(
            {
                "x": x[dg * _BPC:(dg + 1) * _BPC].reshape(_T, _D),
                "w": np.ascontiguousarray(w[:, fg * _FL:(fg + 1) * _FL]),
                "wslice": np.ascontiguousarray(w[core * _WR:(core + 1) * _WR, :]),
            }
        )
    res = run_bass_kernel_spmd(nc, in_maps, list(range(_N_CORES)))
    global _last_results
    _last_results = res
    out = np.empty((_BATCH, _SEQ, _F), dtype=np.float32)
    for core in range(_N_CORES):
        dg, fg = divmod(core, _FG)
        o = res.results[core]["out"].reshape(_BPC, _SEQ, _FL)
        out[dg * _BPC:(dg + 1) * _BPC, :, fg * _FL:(fg + 1) * _FL] = o
    return out


_last_results = None


# revision 16
# speedup vs baseline: 1.0248x; 1.0248x over previous
"""BitLinear inference kernel for Trainium2, SPMD over 8 NeuronCores.

Reference computation (x [B, T, D] f32, kernel [D, F] f32):
  x_norm  = x * rsqrt(mean(x^2, -1) + 1e-5)
  x_scale = 127 / clip(max|x_norm|, 1e-5)          (per row)
  x_quant = round(x_norm * x_scale).clip(-128,127) / x_scale
  w_scale = mean|kernel|.clip(1e-5)
  w_quant = sign(kernel - mean(kernel)) * w_scale
  out     = (x_quant @ w_quant) / w_scale / x_scale

Algebra: w_scale cancels exactly, and with
  q    = round(x * 127/max|x|)   (row scale cancels inside the quant)
  s    = sign(kernel - mean(kernel))  (+-1)
  out  = (q @ s) * (max|x|^2 / (mean(x^2)+1e-5)) / 127^2   per row.
q and s are exactly representable in bf16 and the PE accumulates in
fp32, so the matmul is exact integer arithmetic.

Sharding: 2D grid, 4 data-groups x 2 feature-groups.  Core (dg, fg)
computes batches [2*dg, 2*dg+2) for output features [2048*fg, 2048*(fg+1)).
Its sign slice s[:, fslice] is [4096, 2048] bf16 = 128 KiB/partition,
fully resident in SBUF: produced once by the Activation engine straight
from streamed w columns (no DRAM round-trip, no re-reads).  The global
w mean comes from per-core disjoint 512-row partial column sums +
a tiny AllReduce, so sign production starts ~90us in.  After that the
kernel is a single sweep over 64 token blocks, PE-bound end to end.
"""

import re
from contextlib import ExitStack

import numpy as np

import concourse.bass as bass
import concourse.mybir as mybir
import concourse.tile as tile
from concourse.tile import ScopedClock, VectorClock


# ---------------------------------------------------------------------------
# The walrus build in this container only accepts a single sync-wait per
# Drain instruction; TileContext's tail drain carries one wait per live
# semaphore.  Split it into one drain per semaphore.
# ---------------------------------------------------------------------------
def _drain_and_barrier_split(self, tick_clock, wait_clock):
    m = re.search(r"VectorClock\(\[([^\]]*)\]\)", repr(tick_clock.global_clock))
    vals = [int(v) for v in m.group(1).split(",")]
    procs = [(i, v) for i, v in enumerate(vals) if v > 0]
    for i, v in procs or [(0, 0)]:
        sub = VectorClock()
        sub.require_at_least(i, v)
        drain_inst = self.nc.sync.drain()
        wait_clock.add_sem_waits(drain_inst.ins, ScopedClock({None: sub}))

    self.nc.all_engine_barrier()
    assert self.sems is not None
    popped = self.nc._tile_sem_poison_stack.pop()
    assert popped is self._sem_poison
    self.nc.clear_and_free_semaphores(list(self.sems.allocated().values()))
    self.nc.all_engine_barrier()


def install_drain_patch():
    tile.TileContext._drain_and_barrier = _drain_and_barrier_split


def split_multi_waits(nc: bass.Bass, max_waits: int = 1):
    """The walrus in this container accepts at most one sync-wait per
    instruction.  Hoist extra waits onto NoOps injected just before the
    instruction on the same engine (engines execute their stream in order,
    so waiting on A then B sequentially == waiting on both)."""
    n_split = 0
    for fn in nc.m.functions:
        for bb in fn.blocks:
            insts = bb.instructions
            if not any(
                ins.sync_info is not None and len(ins.sync_info.on_wait or []) > max_waits
                for ins in insts
            ):
                continue
            out = []
            for ins in insts:
                si = ins.sync_info
                if si is not None and len(si.on_wait or []) > max_waits:
                    waits = list(si.on_wait)
                    for j, w in enumerate(waits[:-max_waits]):
                        nop = mybir.InstNoOp(name=f"{ins.name}-wsplit{j}", ins=[], outs=[])
                        nop.engine = ins.engine
                        nop.sync_info = mybir.SyncInfo(on_wait=[w], on_update=[])
                        nc.register_instruction(nop, overwrite=True)
                        out.append(nop)
                    ins.sync_info = mybir.SyncInfo(
                        on_wait=waits[-max_waits:], on_update=list(si.on_update or [])
                    )
                    n_split += 1
                out.append(ins)
            bb.instructions = out
    return n_split


MAGIC = float(1.5 * 2.0**23)  # keeps v+MAGIC in [2^23, 2^24) for |v| <= 2^22 -> RNE to integer
F32 = mybir.dt.float32
BF16 = mybir.dt.bfloat16
P = 128


def build_bitlinear(nc: bass.Bass, T=8192, D=4096, FL=2048, FC=512, WR=512, world=8):
    """Per-core program: x [T, D] f32, w [D, FL] f32 (this core's feature
    column slice), wslice [WR, D] f32 (this core's disjoint row slice of the
    full kernel, for the global-mean partial sum) -> out [T, FL] f32.

    The full [D, FL] sign slice stays resident in SBUF (bf16, 128
    KiB/partition); the main loop is a single sweep over T/128 token
    blocks with 4 PSUM banks of 512 features each accumulating over
    D/128 = 32 contraction slices."""
    AF = mybir.ActivationFunctionType
    KB = D // P          # contraction slices
    NTB = T // P         # 128-token blocks
    NFC = FL // FC       # feature chunks (PSUM-bank sized)
    SUB = 512            # bn_stats subgroup width
    NSUB = D // SUB
    WRB = WR // P        # row-blocks in wslice

    x_in = nc.dram_tensor("x", [T, D], F32, kind="ExternalInput")
    w_in = nc.dram_tensor("w", [D, FL], F32, kind="ExternalInput")
    ws_in = nc.dram_tensor("wslice", [WR, D], F32, kind="ExternalInput")
    out = nc.dram_tensor("out", [T, FL], F32, kind="ExternalOutput")

    with tile.TileContext(nc) as tc, ExitStack() as ctx:
        xp = ctx.enter_context(tc.tile_pool(name="xp", bufs=2))
        qbp = ctx.enter_context(tc.tile_pool(name="qbp", bufs=1))
        qtp = ctx.enter_context(tc.tile_pool(name="qtp", bufs=2))
        w2p = ctx.enter_context(tc.tile_pool(name="w2p", bufs=4))
        w3p = ctx.enter_context(tc.tile_pool(name="w3p", bufs=2))
        stg = ctx.enter_context(tc.tile_pool(name="stg", bufs=3))
        st = ctx.enter_context(tc.tile_pool(name="st", bufs=4))
        postp = ctx.enter_context(tc.tile_pool(name="postp", bufs=4))
        singles = ctx.enter_context(tc.tile_pool(name="singles", bufs=1))
        psmm = ctx.enter_context(tc.tile_pool(name="psmm", bufs=7, space="PSUM"))
        psw = ctx.enter_context(tc.tile_pool(name="psw", bufs=1, space="PSUM"))
        dram = ctx.enter_context(tc.tile_pool(name="dram", bufs=1, space="DRAM"))

        # ---- W pass 1: global mean via per-core partial sums + AllReduce ----
        # Contiguous full-width row-block reads (2 MiB DMAs) of this core's
        # disjoint 512-row slice of the full kernel.
        colsum = singles.tile([P, WRB], F32)
        for rb in range(WRB):
            wt = xp.tile([P, D], F32, tag="xt")
            nc.sync.dma_start(out=wt, in_=ws_in[rb * P:(rb + 1) * P, :])
            nc.vector.reduce_sum(
                out=colsum[:, rb : rb + 1], in_=wt, axis=mybir.AxisListType.X
            )
        if world > 1:
            cc_in = dram.tile([P, WRB], F32, name="cc_in")
            cc_out = dram.tile([P, WRB], F32, name="cc_out", addr_space="Shared")
            nc.gpsimd.dma_start(out=cc_in[:, :], in_=colsum)
            nc.gpsimd.collective_compute(
                "AllReduce",
                mybir.AluOpType.add,
                replica_groups=[list(range(world))],
                ins=[cc_in[:, :]],
                outs=[cc_out[:, :]],
            )
            colsum_all = st.tile([P, WRB], F32)
            nc.gpsimd.dma_start(out=colsum_all, in_=cc_out[:, :])
        else:
            colsum_all = colsum
        rowsum = st.tile([P, 1], F32)
        nc.vector.reduce_sum(out=rowsum, in_=colsum_all, axis=mybir.AxisListType.X)
        ones_sq = singles.tile([P, P], F32)
        nc.vector.memset(ones_sq, 1.0)
        magic_bias = singles.tile([P, 1], F32)
        nc.vector.memset(magic_bias, MAGIC)
        # single matmul reduces the partition dim AND broadcasts the total to
        # all 128 partitions: out[m] = sum_k ones[k,m] * rowsum[k]
        ps_bc = psw.tile([P, 1], F32)
        nc.tensor.matmul(ps_bc, lhsT=ones_sq, rhs=rowsum, start=True, stop=True)
        neg_wmean = singles.tile([P, 1], F32)
        nc.scalar.activation(neg_wmean, ps_bc, AF.Copy, bias=0.0, scale=-1.0 / (D * D))

        # ---- sign production: s = sign(w - mean), straight into SBUF ----
        # One persistent bf16 tile per 512-wide feature chunk.  The w column
        # reads (2 KiB contiguous per row) ride the ACT HWDGE ring and have no
        # dependency on the mean, so they prefetch from t=0; the Sign
        # activations fire as soon as the AllReduced mean lands.  Chunk 0
        # completes first so the first token block's matmuls start while
        # chunks 1-3 are still being produced.
        s_sb = [
            singles.tile([P, KB, FC], BF16, tag=f"ssb{fc}", name=f"ssb{fc}")
            for fc in range(NFC)
        ]
        # ACT produces chunks 0..NFC-2 (Sign activation); the otherwise-idle
        # Pool engine produces the last chunk concurrently via
        # b = ((w + -mean) >= 0) then s = 2b - 1.
        for fc in range(NFC - 1):
            for rb in range(KB):
                wt2 = w2p.tile([P, FC], F32, tag="wtile")
                nc.scalar.dma_start(
                    out=wt2, in_=w_in[rb * P:(rb + 1) * P, fc * FC:(fc + 1) * FC]
                )
                nc.scalar.activation(
                    out=s_sb[fc][:, rb, :], in_=wt2, func=AF.Sign,
                    bias=neg_wmean, scale=1.0,
                )
        for fc in range(NFC - 1, NFC):
            for rb in range(KB):
                wt3 = w3p.tile([P, FC], F32, tag="wtile2")
                nc.gpsimd.dma_start(
                    out=wt3, in_=w_in[rb * P:(rb + 1) * P, fc * FC:(fc + 1) * FC]
                )
                nc.gpsimd.tensor_scalar(
                    out=s_sb[fc][:, rb, :], in0=wt3, scalar1=neg_wmean, scalar2=0.0,
                    op0=mybir.AluOpType.add, op1=mybir.AluOpType.is_ge,
                )
                nc.gpsimd.tensor_scalar(
                    out=s_sb[fc][:, rb, :], in0=s_sb[fc][:, rb, :],
                    scalar1=2.0, scalar2=-1.0,
                    op0=mybir.AluOpType.mult, op1=mybir.AluOpType.add,
                )

        # ---- main loop: single sweep over token blocks ----
        # Evacuations are emitted one block LATE (after block tb's pre-chain
        # and before its matmuls): when the DVE/ACT queues reach evac(tb-1),
        # its matmuls have long retired, so the wait never head-of-line
        # blocks the next block's stats/round chain on those engines.
        pending = None  # (psum tiles, post, tb) awaiting evacuation

        def flush_evacs():
            nonlocal pending
            if pending is None:
                return
            pss, ppost, ptb = pending
            for i, (fc, ps) in enumerate(pss):
                so = stg.tile([P, FC], F32, tag="so")
                if i % 2 == 0:
                    nc.vector.tensor_scalar_mul(so, ps, ppost)
                else:
                    nc.scalar.activation(
                        out=so, in_=ps, func=AF.Copy, bias=0.0, scale=ppost
                    )
                nc.sync.dma_start(
                    out=out[ptb * P:(ptb + 1) * P, fc * FC:(fc + 1) * FC], in_=so
                )
            pending = None

        for tb in range(NTB):
            xt = xp.tile([P, D], F32, tag="xt")
            nc.sync.dma_start(out=xt, in_=x_in[tb * P:(tb + 1) * P, :])

            # mean(x^2) via bn_stats (no main output needed)
            stats6 = st.tile([P, NSUB, 6], F32)
            for i in range(NSUB):
                nc.vector.bn_stats(out=stats6[:, i, :], in_=xt[:, i * SUB:(i + 1) * SUB])
            mv = st.tile([P, 2], F32)
            nc.vector.bn_aggr(out=mv, in_=stats6)
            msq = st.tile([P, 1], F32)
            nc.vector.tensor_mul(msq, mv[:, 0:1], mv[:, 0:1])
            v0 = st.tile([P, 1], F32)
            nc.vector.tensor_add(v0, msq, mv[:, 1:2])
            v1 = st.tile([P, 1], F32)
            nc.vector.tensor_scalar_add(v1, v0, 1e-5)
            r2 = st.tile([P, 1], F32)
            nc.vector.reciprocal(r2, v1)

            am = st.tile([P, 1], F32)
            nc.vector.tensor_reduce(
                out=am, in_=xt, axis=mybir.AxisListType.X,
                op=mybir.AluOpType.max, apply_absolute_value=True,
            )
            am2 = st.tile([P, 1], F32)
            nc.vector.tensor_mul(am2, am, am)
            a2 = st.tile([P, 1], F32)
            nc.vector.tensor_mul(a2, am2, r2)
            post = postp.tile([P, 1], F32)
            nc.vector.tensor_scalar(
                out=post, in0=a2, scalar1=1e-10, scalar2=1.0 / (127.0 * 127.0),
                op0=mybir.AluOpType.max, op1=mybir.AluOpType.mult,
            )
            w1 = st.tile([P, 1], F32)
            nc.vector.tensor_scalar(
                out=w1, in0=am, scalar1=1e-30, scalar2=1.0 / 127.0,
                op0=mybir.AluOpType.max, op1=mybir.AluOpType.mult,
            )
            cc = st.tile([P, 1], F32)
            nc.vector.reciprocal(cc, w1)

            # q = round(x * c) via the magic-number trick (RNE)
            nc.scalar.activation(out=xt, in_=xt, func=AF.Identity, bias=magic_bias, scale=cc)
            qb = qbp.tile([P, D], BF16)
            nc.vector.tensor_scalar_add(qb, xt, -MAGIC)

            # one batched xbar transpose per block: qT[p,k,t] = qb[t, k*P+p].
            # Stays on the ACT ring: moving it to the sync ring puts its
            # wait-on-sub entry ahead of the next x load (in-order ring) and
            # inflates the steady-state cadence.
            qT = qtp.tile([P, KB, P], BF16, tag="qT")
            nc.scalar.dma_start_transpose(out=qT, in_=qb[:, :])

            # previous block's evacuations go in front of this block's
            # matmuls; their waits are already satisfied
            flush_evacs()

            # chunk order matches production readiness: 0 (ACT first), 3
            # (Pool, concurrent), then 1, 2 (ACT) — so early blocks always
            # have a ready chunk to chew on while production drains
            pss = []
            for fc in (0, NFC - 1, 1, 2) if NFC == 4 else range(NFC):
                ps = psmm.tile([P, FC], F32, tag="ps", name="ps")
                for k in range(KB):
                    nc.tensor.matmul(
                        ps,
                        lhsT=qT[:, k, :],
                        rhs=s_sb[fc][:, k, :],
                        start=(k == 0),
                        stop=(k == KB - 1),
                    )
                pss.append((fc, ps))
            pending = (pss, post, tb)
        flush_evacs()
    return nc


_N_CORES = 8
_BATCH = 8
_SEQ = 4096
_D = 4096
_F = 4096
_DG = 4            # data groups (batch pairs)
_FG = 2            # feature groups
_BPC = _BATCH // _DG      # batches per core
_T = _BPC * _SEQ          # tokens per core
_FL = _F // _FG           # features per core
_WR = _D // _N_CORES      # wslice rows per core


def _ensure_axon_hooks_module():
    """bass_utils imports antenv.axon_hooks when BASS_TRACE is set; the
    module is absent in this image.  Provide a stub so tracing degrades
    gracefully instead of crashing (a real hook may already be installed)."""
    import sys
    import types

    try:
        import antenv.axon_hooks  # noqa: F401
    except ImportError:
        mod = types.ModuleType("antenv.axon_hooks")
        mod._hook = None
        mod.set_axon_ntff_profile_hook = lambda h: setattr(mod, "_hook", h)
        mod.get_axon_ntff_profile_hook = lambda: mod._hook
        sys.modules["antenv.axon_hooks"] = mod


def kernel(x: np.ndarray, kernel: np.ndarray) -> np.ndarray:
    from concourse.bass_utils import run_bass_kernel_spmd

    _ensure_axon_hooks_module()
    install_drain_patch()
    nc = bass.Bass()
    build_bitlinear(nc, T=_T, D=_D, FL=_FL, FC=512, WR=_WR, world=_N_CORES)
    split_multi_waits(nc)

    x = np.ascontiguousarray(np.asarray(x, dtype=np.float32))
    w = np.ascontiguousarray(np.asarray(kernel, dtype=np.float32))
    assert x.shape == (_BATCH, _SEQ, _D) and w.shape == (_D, _F)

    in_maps = []
    for core in range(_N_CORES):
        dg, fg = divmod(core, _FG)
        in_maps.append(
            {
                "x": x[dg * _BPC:(dg + 1) * _BPC].reshape(_T, _D),
                "w": np.ascontiguousarray(w[:, fg * _FL:(fg + 1) * _FL]),
                "wslice": np.ascontiguousarray(w[core * _WR:(core + 1) * _WR, :]),
            }
        )
    res = run_bass_kernel_spmd(nc, in_maps, list(range(_N_CORES)))
    global _last_results
    _last_results = res
    out = np.empty((_BATCH, _SEQ, _F), dtype=np.float32)
    for core in range(_N_CORES):
        dg, fg = divmod(core, _FG)
        o = res.results[core]["out"].reshape(_BPC, _SEQ, _FL)
        out[dg * _BPC:(dg + 1) * _BPC, :, fg * _FL:(fg + 1) * _FL] = o
    return out


_last_results = None
